# revision 17
# baseline (speedup 1.0000x reference)
"""Trainium2 Bass kernel for nn_Attention_18760417149505.

Reference computation (per problem):
  q/k/v = (x @ W.T + b).reshape(B, H, S, dk)      # flat reshape, NOT head-split
  scores = q @ k.T ; t = (scores*SCALE) @ v ; attn = softmax(t, axis=-1)
  out = ((attn.reshape(B,S,D) @ Wo.T + bo) @ Wf.T + bf)

Key algebraic property: softmax comes AFTER both score matmuls, so the chain
is linear and associative:  (q @ k.T * SCALE) @ v == q @ (SCALE * k.T @ v).
Per (batch, head) we only need the 64x64 Gram matrix G = SCALE * k.T @ v.

Sharding: the flat reshape makes head h own flat rows [2048h, 2048(h+1)) of
the [B*24576, 64] flat view, which equals rows [512c, 512(c+1)) of the
[4096, 768] (B*S, D) matrix for head-triple c. Core c gets x rows
[512c, 512(c+1)) and heads {3c, 3c+1, 3c+2} — fully local, no collectives.
Within a core the local flat index rho = 12*s + g (s local row, g column
group of 64) with head l = rho // 2048 — identical tables on every core
(512*12 == 3*2048).

All projections are computed transposed (o on partitions): Y.T = W @ x.T with
lhsT = W.T chunks, rhs = x.T chunks — both host-pretransposed, fp32r
(pre-rounded on host; fp32r streams at bf16 rate for N>=256). The per-head
[t, dk] k/v chunks are carved out of K.T/V.T via PE transpose-mode matmuls
against half-selector matrices (always K=128 at partition base 0 — K=64
row-strip alternation wedges the PE on hardware). The attention chain
(G, q@G, softmax) runs in full fp32.
"""

import numpy as np

import concourse.bass as bass
import concourse.mybir as mybir
import concourse.tile as tile
from concourse import bacc
from concourse.bass_utils import run_bass_kernel_spmd
from concourse.masks import make_identity

F32 = mybir.dt.float32
F32R = mybir.dt.float32r

B, S, D = 2, 2048, 768
H, DK = 12, 64
SCALE = 0.125
NCORES = 8
SLOC = 512          # x rows per core
HLOC = 3            # heads per core
NCH = 24            # T/A chunks per head (12 groups x 2)


def _ceil_div(a, b):
    return -((-a) // b)


def _slabs():
    """Per (head l, group g): local row range [s_lo, s_hi) of the slab."""
    tab = {}
    for l in range(HLOC):
        tot = 0
        for g in range(12):
            s_lo = max(0, _ceil_div(2048 * l - g, 12))
            s_hi = min(SLOC, _ceil_div(2048 * (l + 1) - g, 12))
            tab[(l, g)] = (s_lo, s_hi)
            tot += s_hi - s_lo
        assert tot == 2048, tot
    return tab


SLABS = _slabs()


def _round_fp32r(x):
    x = np.ascontiguousarray(x, np.float32)
    u = x.view(np.uint32).astype(np.uint64)
    low = u & 0xFFF
    u = u >> 12
    up = (low > 0x800) | ((low == 0x800) & ((u & 1) == 1))
    u = (u + up.astype(np.uint64)) << 12
    return u.astype(np.uint32).view(np.float32)


def build_nc(stage=9):
    nc = bacc.Bacc()

    xT = nc.declare_dram_parameter("xT", [D, SLOC], F32R, isOutput=False)
    wqT = nc.declare_dram_parameter("wqT", [D, D], F32R, isOutput=False)
    wkT = nc.declare_dram_parameter("wkT", [D, D], F32R, isOutput=False)
    wvT = nc.declare_dram_parameter("wvT", [D, D], F32R, isOutput=False)
    woT = nc.declare_dram_parameter("woT", [D, D], F32R, isOutput=False)
    wfT = nc.declare_dram_parameter("wfT", [D, D], F32R, isOutput=False)
    # per-partition packed biases: [:, i, j] = b_i[128j+p], i in (q, k, v, o, f)
    bias_po = nc.declare_dram_parameter("bias_po", [128, 5, 6], F32, isOutput=False)
    outT = nc.declare_dram_parameter("outT", [D, SLOC], F32, isOutput=True)

    ACT_ID = mybir.ActivationFunctionType.Identity

    with tile.TileContext(nc) as tc:
        with (
            tc.tile_pool(name="consts", bufs=1) as consts,
            tc.tile_pool(name="qt", bufs=1) as qtp,
            tc.tile_pool(name="gsb", bufs=1) as gsbp,
            tc.tile_pool(name="softmax", bufs=2) as smp,
            tc.tile_pool(name="mt", bufs=1) as mtp,
        ):
            ident = consts.tile([128, 128], F32)
            make_identity(nc, ident)
            bias_sb = consts.tile([128, 5, 6], F32)
            nc.sync.dma_start(out=bias_sb, in_=bias_po[:, :, :])

            qt_sb = [qtp.tile([128, SLOC], F32, tag=f"qt{j}", name=f"qt{j}")
                     for j in range(6)]
            # zero-padded G variants: [:, 0, l, :] = [G_l; 0], [:, 1, l, :] = [0; G_l]
            g_sb = gsbp.tile([128, 2, HLOC, DK], F32)
            mt_sb = [mtp.tile([128, SLOC], F32R, tag=f"mt{k}", name=f"mt{k}")
                     for k in range(6)]

            with tc.tile_pool(name="kvt", bufs=1) as kvtp:
                kc = [kvtp.tile([128, NCH, DK], F32, tag=f"kc{l}", name=f"kc{l}")
                      for l in range(HLOC)]
                vc = [kvtp.tile([128, NCH, DK], F32, tag=f"vc{l}", name=f"vc{l}")
                      for l in range(HLOC)]

                with tc.tile_pool(name="ktv", bufs=1) as ktvp:
                    kt_sb = [ktvp.tile([128, SLOC], F32, tag=f"kt{j}", name=f"kt{j}")
                             for j in range(6)]
                    vt_sb = [ktvp.tile([128, SLOC], F32, tag=f"vt{j}", name=f"vt{j}")
                             for j in range(6)]

                    with tc.tile_pool(name="xw", bufs=1) as xwp:
                        xT_sb = [xwp.tile([128, SLOC], F32R, tag=f"x{k}", name=f"x{k}")
                                 for k in range(6)]
                        wqT_sb = [xwp.tile([128, D], F32R, tag=f"wq{k}", name=f"wq{k}")
                                  for k in range(6)]
                        wkT_sb = [xwp.tile([128, D], F32R, tag=f"wk{k}", name=f"wk{k}")
                                  for k in range(6)]
                        wvT_sb = [xwp.tile([128, D], F32R, tag=f"wv{k}", name=f"wv{k}")
                                  for k in range(6)]
                        # K-projection inputs first (they gate the pipeline),
                        # V/Q weights on the software DGE in parallel
                        for k in range(6):
                            nc.sync.dma_start(out=xT_sb[k],
                                              in_=xT[128 * k:128 * (k + 1), :])
                            nc.sync.dma_start(out=wkT_sb[k],
                                              in_=wkT[128 * k:128 * (k + 1), :])
                        for k in range(6):
                            nc.gpsimd.dma_start(out=wvT_sb[k],
                                                in_=wvT[128 * k:128 * (k + 1), :])
                        for k in range(6):
                            nc.gpsimd.dma_start(out=wqT_sb[k],
                                                in_=wqT[128 * k:128 * (k + 1), :])

                        def project(bi, w_sb, dst, ppj):
                            # Y.T = W @ x.T (o on partitions)
                            for j in range(6):
                                ps = ppj.tile([128, 512], F32, tag="pj")
                                for k in range(6):
                                    nc.tensor.matmul(
                                        ps,
                                        w_sb[k][:, 128 * j:128 * (j + 1)],
                                        xT_sb[k],
                                        start=(k == 0), stop=(k == 5),
                                    )
                                nc.scalar.activation(
                                    dst[j], ps, ACT_ID,
                                    bias=bias_sb[:, bi, j:j + 1],
                                )

                        if stage >= 1:
                            with tc.tile_pool(name="ppj", bufs=4,
                                              space="PSUM") as ppj:
                                project(1, wkT_sb, kt_sb, ppj)
                                project(2, wvT_sb, vt_sb, ppj)

                                # carve per-head [t, dk] chunks of k/v out of
                                # K.T/V.T via PE transposes (K=128, base 0)
                                if stage >= 2:
                                    with tc.tile_pool(name="pptr2", bufs=4,
                                                      space="PSUM") as pptr2:
                                        for l in range(HLOC):
                                            for g in range(12):
                                                s_lo, s_hi = SLABS[(l, g)]
                                                h2 = g % 2
                                                for c in (0, 1):
                                                    s0 = s_lo + 128 * c
                                                    s1 = min(s_hi,
                                                             s_lo + 128 * (c + 1))
                                                    m = s1 - s0
                                                    for src, dst in ((kt_sb, kc),
                                                                     (vt_sb, vc)):
                                                        trp = pptr2.tile(
                                                            [128, 128], F32,
                                                            tag="tr2")
                                                        nc.tensor.transpose(
                                                            trp[0:m, :],
                                                            src[g // 2][:, s0:s1],
                                                            ident,
                                                        )
                                                        nc.any.tensor_copy(
                                                            dst[l][0:m, 2 * g + c, :],
                                                            trp[0:m,
                                                                64 * h2:64 * h2 + 64],
                                                        )
                                project(0, wqT_sb, qt_sb, ppj)

                # G = SCALE * k.T @ v per head (fp32, ragged K accumulation)
                if stage >= 3:
                    nc.vector.memset(g_sb, 0.0)
                    with tc.tile_pool(name="ppg", bufs=1, space="PSUM") as ppg:
                        gps = ppg.tile([DK, HLOC, DK], F32)
                        for l in range(HLOC):
                            pieces = []
                            for g in range(12):
                                s_lo, s_hi = SLABS[(l, g)]
                                L = s_hi - s_lo
                                pieces.append((2 * g, min(128, L)))
                                if L > 128:
                                    pieces.append((2 * g + 1, L - 128))
                            for i, (c, kk) in enumerate(pieces):
                                nc.tensor.matmul(
                                    gps[:, l, :],
                                    kc[l][0:kk, c, :],
                                    vc[l][0:kk, c, :],
                                    start=(i == 0), stop=(i == len(pieces) - 1),
                                )
                            # release each head's G as soon as it is done
                            nc.vector.tensor_scalar_mul(
                                g_sb[0:64, 0, l, :], gps[:, l, :], SCALE)
                            # odd-group variant lives in partitions 64..127
                            nc.sync.dma_start(out=g_sb[64:128, 1, l, :],
                                              in_=g_sb[0:64, 0, l, :])

            with tc.tile_pool(name="wof", bufs=1) as wofp:
                woT_sb = [wofp.tile([128, D], F32R, tag=f"wo{k}", name=f"wo{k}")
                          for k in range(6)]
                wfT_sb = [wofp.tile([128, D], F32R, tag=f"wf{k}", name=f"wf{k}")
                          for k in range(6)]
                for k in range(6):
                    nc.sync.dma_start(out=woT_sb[k], in_=woT[128 * k:128 * (k + 1), :])
                    nc.gpsimd.dma_start(out=wfT_sb[k],
                                        in_=wfT[128 * k:128 * (k + 1), :])

                # T = q @ G per head -> psum [128, NCH, DK]; softmax over dk
                if stage >= 4:
                    with (
                        tc.tile_pool(name="ppt", bufs=2, space="PSUM") as ppt,
                        tc.tile_pool(name="pptr", bufs=2, space="PSUM") as pptr,
                    ):
                        for l in range(HLOC):
                            tps = ppt.tile([128, NCH, DK], F32, tag="T", name=f"T{l}")
                            for g in range(12):
                                s_lo, s_hi = SLABS[(l, g)]
                                for c in (0, 1):
                                    s0 = s_lo + 128 * c
                                    col0 = min(s0, SLOC - 128)
                                    nc.tensor.matmul(
                                        tps[:, 2 * g + c, :],
                                        qt_sb[g // 2][:, col0:col0 + 128],
                                        g_sb[:, g % 2, l, :],
                                        start=True, stop=True,
                                    )
                            # softmax over the dk axis
                            negmax = smp.tile([128, NCH], F32, tag="nm", name=f"nm{l}")
                            nc.vector.reduce_max(negmax, tps,
                                                 axis=mybir.AxisListType.X,
                                                 negate=True)
                            av = smp.tile([128, NCH, DK], F32, tag="A", name=f"A{l}")
                            nm_b = bass.AP(tensor=negmax.tensor, offset=negmax.offset,
                                           ap=[negmax.ap[0], negmax.ap[1], [0, DK]])
                            nc.vector.tensor_add(av, tps, nm_b)
                            nc.scalar.activation(av, av,
                                                 mybir.ActivationFunctionType.Exp)
                            sm = smp.tile([128, NCH], F32, tag="sm", name=f"sm{l}")
                            nc.vector.reduce_sum(sm, av, axis=mybir.AxisListType.X)
                            inv = smp.tile([128, NCH], F32, tag="inv", name=f"inv{l}")
                            nc.vector.reciprocal(inv, sm)
                            inv_b = bass.AP(tensor=inv.tensor, offset=inv.offset,
                                            ap=[inv.ap[0], inv.ap[1], [0, DK]])
                            nc.vector.tensor_mul(av, av, inv_b)

                            # transpose A chunks into M.T tiles (fp32r)
                            if stage >= 5:
                                if True:
                                    for g in range(12):
                                        s_lo, s_hi = SLABS[(l, g)]
                                        h2 = (g % 2) * 64
                                        for c in (0, 1):
                                            s0 = s_lo + 128 * c
                                            s1 = min(s_hi, s_lo + 128 * (c + 1))
                                            col0 = min(s0, SLOC - 128)
                                            dlt = s0 - col0
                                            trp = pptr.tile([128, 128], F32, tag="tr",
                                                            name=f"tr{l}{g}{c}")
                                            if h2 == 0:
                                                nc.tensor.transpose(
                                                    trp[0:64, :],
                                                    av[:, 2 * g + c, :],
                                                    ident,
                                                )
                                            else:
                                                # transpose-mode psum out must
                                                # start at partition 0; emulate
                                                # via A.T @ I
                                                nc.tensor.matmul(
                                                    trp[64:128, :],
                                                    av[:, 2 * g + c, :],
                                                    ident,
                                                    start=True, stop=True,
                                                )
                                            nc.any.tensor_copy(
                                                mt_sb[g // 2][h2:h2 + 64, s0:s1],
                                                trp[h2:h2 + 64, dlt:dlt + (s1 - s0)],
                                            )

                # output projections: O.T = Wo @ M, OUT.T = Wf @ O (fp32r)
                if stage >= 6:
                    with (
                        tc.tile_pool(name="ot", bufs=1) as otp,
                        tc.tile_pool(name="ppo", bufs=3, space="PSUM") as ppo,
                    ):
                        ot_sb = [otp.tile([128, SLOC], F32R, tag=f"ot{j}",
                                          name=f"ot{j}") for j in range(6)]
                        out_sb = [otp.tile([128, SLOC], F32, tag=f"ou{j}",
                                           name=f"ou{j}") for j in range(6)]
                        for j in range(6):
                            ps = ppo.tile([128, 512], F32, tag="po")
                            for k in range(6):
                                nc.tensor.matmul(
                                    ps, woT_sb[k][:, 128 * j:128 * (j + 1)], mt_sb[k],
                                    start=(k == 0), stop=(k == 5),
                                )
                            nc.scalar.activation(
                                ot_sb[j], ps, ACT_ID, bias=bias_sb[:, 3, j:j + 1],
                            )
                        for j in range(6):
                            ps = ppo.tile([128, 512], F32, tag="po")
                            for k in range(6):
                                nc.tensor.matmul(
                                    ps, wfT_sb[k][:, 128 * j:128 * (j + 1)], ot_sb[k],
                                    start=(k == 0), stop=(k == 5),
                                )
                            nc.scalar.activation(
                                out_sb[j], ps, ACT_ID, bias=bias_sb[:, 4, j:j + 1],
                            )
                            nc.sync.dma_start(out=outT[128 * j:128 * (j + 1), :],
                                              in_=out_sb[j])

    nc.finalize()
    return nc


_NC_CACHE = None


def make_in_maps(x, Wq, bq, Wk, bk, Wv, bv, Wo, bo, Wf, bf):
    xf = np.ascontiguousarray(np.asarray(x, np.float32).reshape(B * S, D))
    shared = {
        "wqT": _round_fp32r(np.asarray(Wq, np.float32).T),
        "wkT": _round_fp32r(np.asarray(Wk, np.float32).T),
        "wvT": _round_fp32r(np.asarray(Wv, np.float32).T),
        "woT": _round_fp32r(np.asarray(Wo, np.float32).T),
        "wfT": _round_fp32r(np.asarray(Wf, np.float32).T),
        "bias_po": np.stack(
            [np.asarray(b, np.float32).reshape(6, 128).T
             for b in (bq, bk, bv, bo, bf)],
            axis=1,
        ).copy(),
    }
    in_maps = []
    for c in range(NCORES):
        m = dict(shared)
        m["xT"] = _round_fp32r(xf[SLOC * c:SLOC * (c + 1), :].T)
        in_maps.append(m)
    return in_maps


def kernel(**inputs):
    global _NC_CACHE
    if _NC_CACHE is None:
        _NC_CACHE = build_nc()
    nc = _NC_CACHE
    in_maps = make_in_maps(**inputs)
    res = run_bass_kernel_spmd(nc, in_maps, list(range(NCORES)))
    out = np.empty((B * S, D), np.float32)
    for c in range(NCORES):
        out[SLOC * c:SLOC * (c + 1), :] = res.results[c]["outT"].T
    return out.reshape(B, S, D)


# revision 24
# speedup vs baseline: 1.0464x; 1.0464x over previous
"""Trainium2 Bass kernel for nn_Attention_18760417149505.

Reference computation (per problem):
  q/k/v = (x @ W.T + b).reshape(B, H, S, dk)      # flat reshape, NOT head-split
  scores = q @ k.T ; t = (scores*SCALE) @ v ; attn = softmax(t, axis=-1)
  out = ((attn.reshape(B,S,D) @ Wo.T + bo) @ Wf.T + bf)

Key algebraic property: softmax comes AFTER both score matmuls, so the chain
is linear and associative:  (q @ k.T * SCALE) @ v == q @ (SCALE * k.T @ v).
Per (batch, head) we only need the 64x64 Gram matrix G = SCALE * k.T @ v.

Sharding: the flat reshape makes head h own flat rows [2048h, 2048(h+1)) of
the [B*24576, 64] flat view, which equals rows [512c, 512(c+1)) of the
[4096, 768] (B*S, D) matrix for head-triple c. Core c gets x rows
[512c, 512(c+1)) and heads {3c, 3c+1, 3c+2} — fully local, no collectives.
Within a core the local flat index rho = 12*s + g (s local row, g column
group of 64) with head l = rho // 2048 — identical tables on every core
(512*12 == 3*2048).

All projections are computed transposed (o on partitions): Y.T = W @ x.T with
lhsT = W.T chunks, rhs = x.T chunks — both host-pretransposed, fp32r
(pre-rounded on host; fp32r streams at bf16 rate for N>=256). The per-head
[t, dk] k/v chunks are carved out of K.T/V.T via PE transpose-mode matmuls
against half-selector matrices (always K=128 at partition base 0 — K=64
row-strip alternation wedges the PE on hardware). The attention chain
(G, q@G, softmax) runs in full fp32.
"""

import numpy as np

import concourse.bass as bass
import concourse.mybir as mybir
import concourse.tile as tile
from concourse import bacc
from concourse.bass_utils import run_bass_kernel_spmd
from concourse.masks import make_identity

F32 = mybir.dt.float32
F32R = mybir.dt.float32r

B, S, D = 2, 2048, 768
H, DK = 12, 64
SCALE = 0.125
NCORES = 8
SLOC = 512          # x rows per core
HLOC = 3            # heads per core
NCH = 24            # T/A chunks per head (12 groups x 2)


def _ceil_div(a, b):
    return -((-a) // b)


def _slabs():
    """Per (head l, group g): local row range [s_lo, s_hi) of the slab."""
    tab = {}
    for l in range(HLOC):
        tot = 0
        for g in range(12):
            s_lo = max(0, _ceil_div(2048 * l - g, 12))
            s_hi = min(SLOC, _ceil_div(2048 * (l + 1) - g, 12))
            tab[(l, g)] = (s_lo, s_hi)
            tot += s_hi - s_lo
        assert tot == 2048, tot
    return tab


SLABS = _slabs()


def _round_fp32r(x):
    x = np.ascontiguousarray(x, np.float32)
    u = x.view(np.uint32).astype(np.uint64)
    low = u & 0xFFF
    u = u >> 12
    up = (low > 0x800) | ((low == 0x800) & ((u & 1) == 1))
    u = (u + up.astype(np.uint64)) << 12
    return u.astype(np.uint32).view(np.float32)


def build_nc(stage=9):
    nc = bacc.Bacc()

    xT = nc.declare_dram_parameter("xT", [D, SLOC], F32R, isOutput=False)
    wqT = nc.declare_dram_parameter("wqT", [D, D], F32R, isOutput=False)
    wkT = nc.declare_dram_parameter("wkT", [D, D], F32R, isOutput=False)
    wvT = nc.declare_dram_parameter("wvT", [D, D], F32R, isOutput=False)
    woT = nc.declare_dram_parameter("woT", [D, D], F32R, isOutput=False)
    wfT = nc.declare_dram_parameter("wfT", [D, D], F32R, isOutput=False)
    # per-partition packed biases: [:, i, j] = b_i[128j+p], i in (q, k, v, o, f)
    bias_po = nc.declare_dram_parameter("bias_po", [128, 5, 6], F32, isOutput=False)
    outT = nc.declare_dram_parameter("outT", [D, SLOC], F32, isOutput=True)

    ACT_ID = mybir.ActivationFunctionType.Identity

    with tile.TileContext(nc) as tc:
        with (
            tc.tile_pool(name="consts", bufs=1) as consts,
            tc.tile_pool(name="qt", bufs=1) as qtp,
            tc.tile_pool(name="gsb", bufs=1) as gsbp,
            tc.tile_pool(name="softmax", bufs=4) as smp,
            tc.tile_pool(name="mt", bufs=1) as mtp,
        ):
            ident = consts.tile([128, 128], F32)
            make_identity(nc, ident)
            bias_sb = consts.tile([128, 5, 6], F32)
            nc.sync.dma_start(out=bias_sb, in_=bias_po[:, :, :])

            qt_sb = [qtp.tile([128, SLOC], F32, tag=f"qt{j}", name=f"qt{j}")
                     for j in range(6)]
            # zero-padded G variants: [:, 0, l, :] = [G_l; 0], [:, 1, l, :] = [0; G_l]
            g_sb = gsbp.tile([128, 2, HLOC, DK], F32)
            mt_sb = [mtp.tile([128, SLOC], F32R, tag=f"mt{k}", name=f"mt{k}")
                     for k in range(6)]

            with tc.tile_pool(name="kvt", bufs=1) as kvtp:
                # k at [:, 0, ch, :], v at [:, 1, ch, :]
                kvc = [kvtp.tile([128, 2, NCH, DK], F32, tag=f"kvc{l}",
                                 name=f"kvc{l}") for l in range(HLOC)]

                with tc.tile_pool(name="ktv", bufs=1) as ktvp:
                    kt_sb = [ktvp.tile([128, SLOC], F32, tag=f"kt{j}", name=f"kt{j}")
                             for j in range(6)]
                    vt_sb = [ktvp.tile([128, SLOC], F32, tag=f"vt{j}", name=f"vt{j}")
                             for j in range(6)]

                    with tc.tile_pool(name="xw", bufs=1) as xwp:
                        xT_sb = [xwp.tile([128, SLOC], F32R, tag=f"x{k}", name=f"x{k}")
                                 for k in range(6)]
                        wqT_sb = [xwp.tile([128, D], F32R, tag=f"wq{k}", name=f"wq{k}")
                                  for k in range(6)]
                        wkT_sb = [xwp.tile([128, D], F32R, tag=f"wk{k}", name=f"wk{k}")
                                  for k in range(6)]
                        wvT_sb = [xwp.tile([128, D], F32R, tag=f"wv{k}", name=f"wv{k}")
                                  for k in range(6)]
                        # K-projection inputs first (they gate the pipeline),
                        # V/Q weights on the software DGE in parallel
                        for k in range(6):
                            nc.sync.dma_start(out=xT_sb[k],
                                              in_=xT[128 * k:128 * (k + 1), :])
                            nc.sync.dma_start(out=wkT_sb[k],
                                              in_=wkT[128 * k:128 * (k + 1), :])
                        for k in range(6):
                            nc.gpsimd.dma_start(out=wvT_sb[k],
                                                in_=wvT[128 * k:128 * (k + 1), :])
                        for k in range(6):
                            nc.gpsimd.dma_start(out=wqT_sb[k],
                                                in_=wqT[128 * k:128 * (k + 1), :])

                        def project(bi, w_sb, dst, ppj):
                            # Y.T = W @ x.T (o on partitions)
                            for j in range(6):
                                ps = ppj.tile([128, 512], F32, tag="pj")
                                for k in range(6):
                                    nc.tensor.matmul(
                                        ps,
                                        w_sb[k][:, 128 * j:128 * (j + 1)],
                                        xT_sb[k],
                                        start=(k == 0), stop=(k == 5),
                                    )
                                nc.scalar.activation(
                                    dst[j], ps, ACT_ID,
                                    bias=bias_sb[:, bi, j:j + 1],
                                )

                        if stage >= 1:
                            with tc.tile_pool(name="ppj", bufs=4,
                                              space="PSUM") as ppj:
                                project(1, wkT_sb, kt_sb, ppj)
                                project(2, wvT_sb, vt_sb, ppj)

                                # carve per-head [t, dk] chunks of k/v out of
                                # K.T/V.T via PE transposes (K=128, base 0)
                                if stage >= 2:
                                    with tc.tile_pool(name="pptr2", bufs=4,
                                                      space="PSUM") as pptr2:
                                        for l in range(HLOC):
                                            for g in range(12):
                                                s_lo, s_hi = SLABS[(l, g)]
                                                h2 = g % 2
                                                for c in (0, 1):
                                                    s0 = s_lo + 128 * c
                                                    s1 = min(s_hi,
                                                             s_lo + 128 * (c + 1))
                                                    m = s1 - s0
                                                    trp = pptr2.tile(
                                                        [128, 2, 128], F32,
                                                        tag="tr2")
                                                    nc.tensor.transpose(
                                                        trp[0:m, 0, :],
                                                        kt_sb[g // 2][:, s0:s1],
                                                        ident,
                                                    )
                                                    nc.tensor.transpose(
                                                        trp[0:m, 1, :],
                                                        vt_sb[g // 2][:, s0:s1],
                                                        ident,
                                                    )
                                                    nc.any.tensor_copy(
                                                        kvc[l][0:m, :, 2 * g + c, :],
                                                        trp[0:m, :,
                                                            64 * h2:64 * h2 + 64],
                                                    )
                                project(0, wqT_sb, qt_sb, ppj)

                # G = SCALE * k.T @ v per head (fp32, ragged K accumulation)
                if stage >= 3:
                    nc.vector.memset(g_sb, 0.0)
                    with tc.tile_pool(name="ppg", bufs=1, space="PSUM") as ppg:
                        gps = ppg.tile([DK, HLOC, DK], F32)
                        for l in range(HLOC):
                            pieces = []
                            for g in range(12):
                                s_lo, s_hi = SLABS[(l, g)]
                                L = s_hi - s_lo
                                pieces.append((2 * g, min(128, L)))
                                if L > 128:
                                    pieces.append((2 * g + 1, L - 128))
                            for i, (c, kk) in enumerate(pieces):
                                nc.tensor.matmul(
                                    gps[:, l, :],
                                    kvc[l][0:kk, 0, c, :],
                                    kvc[l][0:kk, 1, c, :],
                                    start=(i == 0), stop=(i == len(pieces) - 1),
                                )
                            # release each head's G as soon as it is done
                            nc.vector.tensor_scalar_mul(
                                g_sb[0:64, 0, l, :], gps[:, l, :], SCALE)
                            # odd-group variant lives in partitions 64..127
                            nc.sync.dma_start(out=g_sb[64:128, 1, l, :],
                                              in_=g_sb[0:64, 0, l, :])

            with tc.tile_pool(name="wof", bufs=1) as wofp:
                woT_sb = [wofp.tile([128, D], F32R, tag=f"wo{k}", name=f"wo{k}")
                          for k in range(6)]
                wfT_sb = [wofp.tile([128, D], F32R, tag=f"wf{k}", name=f"wf{k}")
                          for k in range(6)]
                for k in range(6):
                    nc.sync.dma_start(out=woT_sb[k], in_=woT[128 * k:128 * (k + 1), :])
                    nc.gpsimd.dma_start(out=wfT_sb[k],
                                        in_=wfT[128 * k:128 * (k + 1), :])

                # T = q @ G per head -> psum [128, NCH, DK]; softmax over dk
                if stage >= 4:
                    with (
                        tc.tile_pool(name="ppt", bufs=2, space="PSUM") as ppt,
                        tc.tile_pool(name="pptr", bufs=2, space="PSUM") as pptr,
                    ):
                        NH = NCH // 2
                        for l in range(HLOC):
                            tps = ppt.tile([128, NCH, DK], F32, tag="T", name=f"T{l}")
                            # process in group-halves so the first transposes
                            # start after half a softmax, not a full one
                            for hf in (0, 1):
                                for g in range(6 * hf, 6 * hf + 6):
                                    s_lo, s_hi = SLABS[(l, g)]
                                    for c in (0, 1):
                                        s0 = s_lo + 128 * c
                                        col0 = min(s0, SLOC - 128)
                                        nc.tensor.matmul(
                                            tps[:, 2 * g + c, :],
                                            qt_sb[g // 2][:, col0:col0 + 128],
                                            g_sb[:, g % 2, l, :],
                                            start=True, stop=True,
                                        )
                                th = tps[:, 12 * hf:12 * hf + 12, :]
                                # softmax over the dk axis
                                negmax = smp.tile([128, NH], F32, tag="nm",
                                                  name=f"nm{l}{hf}")
                                nc.vector.reduce_max(negmax, th,
                                                     axis=mybir.AxisListType.X,
                                                     negate=True)
                                av = smp.tile([128, NH, DK], F32, tag="A",
                                              name=f"A{l}{hf}")
                                nm_b = bass.AP(tensor=negmax.tensor,
                                               offset=negmax.offset,
                                               ap=[negmax.ap[0], negmax.ap[1],
                                                   [0, DK]])
                                nc.vector.tensor_add(av, th, nm_b)
                                nc.scalar.activation(
                                    av, av, mybir.ActivationFunctionType.Exp)
                                sm = smp.tile([128, NH], F32, tag="sm",
                                              name=f"sm{l}{hf}")
                                nc.vector.reduce_sum(sm, av,
                                                     axis=mybir.AxisListType.X)
                                inv = smp.tile([128, NH], F32, tag="inv",
                                               name=f"inv{l}{hf}")
                                nc.vector.reciprocal(inv, sm)
                                inv_b = bass.AP(tensor=inv.tensor, offset=inv.offset,
                                                ap=[inv.ap[0], inv.ap[1], [0, DK]])
                                nc.vector.tensor_mul(av, av, inv_b)

                                # transpose A chunks into M.T tiles (fp32r)
                                if stage >= 5:
                                    for g in range(6 * hf, 6 * hf + 6):
                                        s_lo, s_hi = SLABS[(l, g)]
                                        h2 = (g % 2) * 64
                                        for c in (0, 1):
                                            s0 = s_lo + 128 * c
                                            s1 = min(s_hi, s_lo + 128 * (c + 1))
                                            col0 = min(s0, SLOC - 128)
                                            dlt = s0 - col0
                                            trp = pptr.tile([128, 128], F32, tag="tr",
                                                            name=f"tr{l}{g}{c}")
                                            if h2 == 0:
                                                nc.tensor.transpose(
                                                    trp[0:64, :],
                                                    av[:, 2 * g + c - 12 * hf, :],
                                                    ident,
                                                )
                                                nc.any.tensor_copy(
                                                    mt_sb[g // 2][0:64, s0:s1],
                                                    trp[0:64, dlt:dlt + (s1 - s0)],
                                                )
                                            else:
                                                # transpose-mode psum out must
                                                # start at partition 0; emulate
                                                # via A.T @ I, emitting only the
                                                # valid output columns
                                                nc.tensor.matmul(
                                                    trp[64:128, 0:s1 - s0],
                                                    av[:, 2 * g + c - 12 * hf, :],
                                                    ident[:, dlt:dlt + (s1 - s0)],
                                                    start=True, stop=True,
                                                )
                                                nc.any.tensor_copy(
                                                    mt_sb[g // 2][64:128, s0:s1],
                                                    trp[64:128, 0:s1 - s0],
                                                )

                # output projections: O.T = Wo @ M, OUT.T = Wf @ O (fp32r)
                if stage >= 6:
                    with (
                        tc.tile_pool(name="ot", bufs=1) as otp,
                        tc.tile_pool(name="ppo", bufs=3, space="PSUM") as ppo,
                    ):
                        ot_sb = [otp.tile([128, SLOC], F32R, tag=f"ot{j}",
                                          name=f"ot{j}") for j in range(6)]
                        out_sb = [otp.tile([128, SLOC], F32, tag=f"ou{j}",
                                           name=f"ou{j}") for j in range(6)]
                        for j in range(6):
                            ps = ppo.tile([128, 512], F32, tag="po")
                            for k in range(6):
                                nc.tensor.matmul(
                                    ps, woT_sb[k][:, 128 * j:128 * (j + 1)], mt_sb[k],
                                    start=(k == 0), stop=(k == 5),
                                )
                            nc.scalar.activation(
                                ot_sb[j], ps, ACT_ID, bias=bias_sb[:, 3, j:j + 1],
                            )
                        for j in range(6):
                            ps = ppo.tile([128, 512], F32, tag="po")
                            for k in range(6):
                                nc.tensor.matmul(
                                    ps, wfT_sb[k][:, 128 * j:128 * (j + 1)], ot_sb[k],
                                    start=(k == 0), stop=(k == 5),
                                )
                            nc.scalar.activation(
                                out_sb[j], ps, ACT_ID, bias=bias_sb[:, 4, j:j + 1],
                            )
                            nc.sync.dma_start(out=outT[128 * j:128 * (j + 1), :],
                                              in_=out_sb[j])

    nc.finalize()
    return nc


_NC_CACHE = None


def make_in_maps(x, Wq, bq, Wk, bk, Wv, bv, Wo, bo, Wf, bf):
    xf = np.ascontiguousarray(np.asarray(x, np.float32).reshape(B * S, D))
    shared = {
        "wqT": _round_fp32r(np.asarray(Wq, np.float32).T),
        "wkT": _round_fp32r(np.asarray(Wk, np.float32).T),
        "wvT": _round_fp32r(np.asarray(Wv, np.float32).T),
        "woT": _round_fp32r(np.asarray(Wo, np.float32).T),
        "wfT": _round_fp32r(np.asarray(Wf, np.float32).T),
        "bias_po": np.stack(
            [np.asarray(b, np.float32).reshape(6, 128).T
             for b in (bq, bk, bv, bo, bf)],
            axis=1,
        ).copy(),
    }
    in_maps = []
    for c in range(NCORES):
        m = dict(shared)
        m["xT"] = _round_fp32r(xf[SLOC * c:SLOC * (c + 1), :].T)
        in_maps.append(m)
    return in_maps


def kernel(**inputs):
    global _NC_CACHE
    if _NC_CACHE is None:
        _NC_CACHE = build_nc()
    nc = _NC_CACHE
    in_maps = make_in_maps(**inputs)
    res = run_bass_kernel_spmd(nc, in_maps, list(range(NCORES)))
    out = np.empty((B * S, D), np.float32)
    for c in range(NCORES):
        out[SLOC * c:SLOC * (c + 1), :] = res.results[c]["outT"].T
    return out.reshape(B, S, D)


# revision 25
# speedup vs baseline: 1.1279x; 1.0780x over previous
"""Trainium2 Bass kernel for nn_Attention_18760417149505.

Reference computation (per problem):
  q/k/v = (x @ W.T + b).reshape(B, H, S, dk)      # flat reshape, NOT head-split
  scores = q @ k.T ; t = (scores*SCALE) @ v ; attn = softmax(t, axis=-1)
  out = ((attn.reshape(B,S,D) @ Wo.T + bo) @ Wf.T + bf)

Key algebraic property: softmax comes AFTER both score matmuls, so the chain
is linear and associative:  (q @ k.T * SCALE) @ v == q @ (SCALE * k.T @ v).
Per (batch, head) we only need the 64x64 Gram matrix G = SCALE * k.T @ v.

Sharding: the flat reshape makes head h own flat rows [2048h, 2048(h+1)) of
the [B*24576, 64] flat view, which equals rows [512c, 512(c+1)) of the
[4096, 768] (B*S, D) matrix for head-triple c. Core c gets x rows
[512c, 512(c+1)) and heads {3c, 3c+1, 3c+2} — fully local, no collectives.
Within a core the local flat index rho = 12*s + g (s local row, g column
group of 64) with head l = rho // 2048 — identical tables on every core
(512*12 == 3*2048).

All projections are computed transposed (o on partitions): Y.T = W @ x.T with
lhsT = W.T chunks, rhs = x.T chunks — both host-pretransposed, fp32r
(pre-rounded on host; fp32r streams at bf16 rate for N>=256). The per-head
[t, dk] k/v chunks are carved out of K.T/V.T via PE transpose-mode matmuls
against half-selector matrices (always K=128 at partition base 0 — K=64
row-strip alternation wedges the PE on hardware). The attention chain
(G, q@G, softmax) runs in full fp32.
"""

import numpy as np

import concourse.bass as bass
import concourse.mybir as mybir
import concourse.tile as tile
from concourse import bacc
from concourse.bass_utils import run_bass_kernel_spmd
from concourse.masks import make_identity

F32 = mybir.dt.float32
F32R = mybir.dt.float32r

B, S, D = 2, 2048, 768
H, DK = 12, 64
SCALE = 0.125
NCORES = 8
SLOC = 512          # x rows per core
HLOC = 3            # heads per core
NCH = 24            # T/A chunks per head (12 groups x 2)


def _ceil_div(a, b):
    return -((-a) // b)


def _slabs():
    """Per (head l, group g): local row range [s_lo, s_hi) of the slab."""
    tab = {}
    for l in range(HLOC):
        tot = 0
        for g in range(12):
            s_lo = max(0, _ceil_div(2048 * l - g, 12))
            s_hi = min(SLOC, _ceil_div(2048 * (l + 1) - g, 12))
            tab[(l, g)] = (s_lo, s_hi)
            tot += s_hi - s_lo
        assert tot == 2048, tot
    return tab


SLABS = _slabs()


def _round_fp32r(x):
    x = np.ascontiguousarray(x, np.float32)
    u = x.view(np.uint32).astype(np.uint64)
    low = u & 0xFFF
    u = u >> 12
    up = (low > 0x800) | ((low == 0x800) & ((u & 1) == 1))
    u = (u + up.astype(np.uint64)) << 12
    return u.astype(np.uint32).view(np.float32)


def build_nc(stage=9):
    nc = bacc.Bacc()

    xT = nc.declare_dram_parameter("xT", [D, SLOC], F32R, isOutput=False)
    wqT = nc.declare_dram_parameter("wqT", [D, D], F32R, isOutput=False)
    wkT = nc.declare_dram_parameter("wkT", [D, D], F32R, isOutput=False)
    wvT = nc.declare_dram_parameter("wvT", [D, D], F32R, isOutput=False)
    woT = nc.declare_dram_parameter("woT", [D, D], F32R, isOutput=False)
    wfT = nc.declare_dram_parameter("wfT", [D, D], F32R, isOutput=False)
    # per-partition packed biases: [:, i, j] = b_i[128j+p], i in (q, k, v, o, f)
    bias_po = nc.declare_dram_parameter("bias_po", [128, 5, 6], F32, isOutput=False)
    outT = nc.declare_dram_parameter("outT", [D, SLOC], F32, isOutput=True)

    ACT_ID = mybir.ActivationFunctionType.Identity

    with tile.TileContext(nc) as tc:
        with (
            tc.tile_pool(name="consts", bufs=1) as consts,
            tc.tile_pool(name="qt", bufs=1) as qtp,
            tc.tile_pool(name="gsb", bufs=1) as gsbp,
            tc.tile_pool(name="softmax", bufs=4) as smp,
            tc.tile_pool(name="mt", bufs=1) as mtp,
        ):
            ident = consts.tile([128, 128], F32)
            make_identity(nc, ident)
            bias_sb = consts.tile([128, 5, 6], F32)
            nc.sync.dma_start(out=bias_sb, in_=bias_po[:, :, :])

            qt_sb = [qtp.tile([128, SLOC], F32, tag=f"qt{j}", name=f"qt{j}")
                     for j in range(6)]
            # zero-padded G variants: [:, 0, l, :] = [G_l; 0], [:, 1, l, :] = [0; G_l]
            g_sb = gsbp.tile([128, 2, HLOC, DK], F32)
            mt_sb = [mtp.tile([128, SLOC], F32R, tag=f"mt{k}", name=f"mt{k}")
                     for k in range(6)]

            with tc.tile_pool(name="kvt", bufs=1) as kvtp:
                # k at [:, 0, ch, :], v at [:, 1, ch, :]
                kvc = [kvtp.tile([128, 2, NCH, DK], F32, tag=f"kvc{l}",
                                 name=f"kvc{l}") for l in range(HLOC)]

                with tc.tile_pool(name="ktv", bufs=1) as ktvp:
                    kt_sb = [ktvp.tile([128, SLOC], F32, tag=f"kt{j}", name=f"kt{j}")
                             for j in range(6)]
                    vt_sb = [ktvp.tile([128, SLOC], F32, tag=f"vt{j}", name=f"vt{j}")
                             for j in range(6)]

                    with tc.tile_pool(name="xw", bufs=1) as xwp:
                        xT_sb = [xwp.tile([128, SLOC], F32R, tag=f"x{k}", name=f"x{k}")
                                 for k in range(6)]
                        wqT_sb = [xwp.tile([128, D], F32R, tag=f"wq{k}", name=f"wq{k}")
                                  for k in range(6)]
                        wkT_sb = [xwp.tile([128, D], F32R, tag=f"wk{k}", name=f"wk{k}")
                                  for k in range(6)]
                        wvT_sb = [xwp.tile([128, D], F32R, tag=f"wv{k}", name=f"wv{k}")
                                  for k in range(6)]
                        # K-projection inputs first (they gate the pipeline),
                        # V/Q weights on the software DGE in parallel
                        for k in range(6):
                            nc.sync.dma_start(out=xT_sb[k],
                                              in_=xT[128 * k:128 * (k + 1), :])
                            nc.sync.dma_start(out=wkT_sb[k],
                                              in_=wkT[128 * k:128 * (k + 1), :])
                        for k in range(6):
                            nc.gpsimd.dma_start(out=wvT_sb[k],
                                                in_=wvT[128 * k:128 * (k + 1), :])
                        for k in range(6):
                            nc.gpsimd.dma_start(out=wqT_sb[k],
                                                in_=wqT[128 * k:128 * (k + 1), :])

                        def project(bi, w_sb, dst, ppj):
                            # Y.T = W @ x.T (o on partitions)
                            for j in range(6):
                                ps = ppj.tile([128, 512], F32, tag="pj")
                                for k in range(6):
                                    nc.tensor.matmul(
                                        ps,
                                        w_sb[k][:, 128 * j:128 * (j + 1)],
                                        xT_sb[k],
                                        start=(k == 0), stop=(k == 5),
                                    )
                                nc.scalar.activation(
                                    dst[j], ps, ACT_ID,
                                    bias=bias_sb[:, bi, j:j + 1],
                                )

                        if stage >= 1:
                            with tc.tile_pool(name="ppj", bufs=4,
                                              space="PSUM") as ppj:
                                project(1, wkT_sb, kt_sb, ppj)
                                project(2, wvT_sb, vt_sb, ppj)

                                # carve per-head [t, dk] chunks of k/v out of
                                # K.T/V.T via PE transposes (K=128, base 0)
                                if stage >= 2:
                                    with tc.tile_pool(name="pptr2", bufs=4,
                                                      space="PSUM") as pptr2:
                                        for l in range(HLOC):
                                            for j in range(6):
                                                # both groups of a pair share
                                                # identical slab bounds
                                                # (2048l - 2j is even, so the
                                                # ceiling never moves g->g+1)
                                                s_lo, s_hi = SLABS[(l, 2 * j)]
                                                assert SLABS[(l, 2 * j + 1)] == (
                                                    s_lo, s_hi)
                                                for c in (0, 1):
                                                    s0 = s_lo + 128 * c
                                                    s1 = min(s_hi,
                                                             s_lo + 128 * (c + 1))
                                                    m = s1 - s0
                                                    trp = pptr2.tile(
                                                        [128, 2, 128], F32,
                                                        tag="tr2")
                                                    nc.tensor.transpose(
                                                        trp[0:m, 0, :],
                                                        kt_sb[j][:, s0:s1],
                                                        ident,
                                                    )
                                                    nc.tensor.transpose(
                                                        trp[0:m, 1, :],
                                                        vt_sb[j][:, s0:s1],
                                                        ident,
                                                    )
                                                    for h2 in (0, 1):
                                                        nc.any.tensor_copy(
                                                            kvc[l][0:m, :,
                                                                   2 * (2 * j + h2)
                                                                   + c, :],
                                                            trp[0:m, :,
                                                                64 * h2:64 * h2
                                                                + 64],
                                                        )
                                project(0, wqT_sb, qt_sb, ppj)

                # G = SCALE * k.T @ v per head (fp32, ragged K accumulation)
                if stage >= 3:
                    nc.vector.memset(g_sb, 0.0)
                    with tc.tile_pool(name="ppg", bufs=1, space="PSUM") as ppg:
                        gps = ppg.tile([DK, HLOC, DK], F32)
                        for l in range(HLOC):
                            pieces = []
                            for g in range(12):
                                s_lo, s_hi = SLABS[(l, g)]
                                L = s_hi - s_lo
                                pieces.append((2 * g, min(128, L)))
                                if L > 128:
                                    pieces.append((2 * g + 1, L - 128))
                            for i, (c, kk) in enumerate(pieces):
                                nc.tensor.matmul(
                                    gps[:, l, :],
                                    kvc[l][0:kk, 0, c, :],
                                    kvc[l][0:kk, 1, c, :],
                                    start=(i == 0), stop=(i == len(pieces) - 1),
                                )
                            # release each head's G as soon as it is done
                            nc.vector.tensor_scalar_mul(
                                g_sb[0:64, 0, l, :], gps[:, l, :], SCALE)
                            # odd-group variant lives in partitions 64..127
                            nc.sync.dma_start(out=g_sb[64:128, 1, l, :],
                                              in_=g_sb[0:64, 0, l, :])

            with tc.tile_pool(name="wof", bufs=1) as wofp:
                woT_sb = [wofp.tile([128, D], F32R, tag=f"wo{k}", name=f"wo{k}")
                          for k in range(6)]
                wfT_sb = [wofp.tile([128, D], F32R, tag=f"wf{k}", name=f"wf{k}")
                          for k in range(6)]
                for k in range(6):
                    nc.sync.dma_start(out=woT_sb[k], in_=woT[128 * k:128 * (k + 1), :])
                    nc.gpsimd.dma_start(out=wfT_sb[k],
                                        in_=wfT[128 * k:128 * (k + 1), :])

                # T = q @ G per head -> psum [128, NCH, DK]; softmax over dk
                if stage >= 4:
                    with (
                        tc.tile_pool(name="ppt", bufs=2, space="PSUM") as ppt,
                        tc.tile_pool(name="pptr", bufs=2, space="PSUM") as pptr,
                    ):
                        NH = NCH // 2
                        for l in range(HLOC):
                            tps = ppt.tile([128, NCH, DK], F32, tag="T", name=f"T{l}")
                            # process in group-halves so the first transposes
                            # start after half a softmax, not a full one
                            for hf in (0, 1):
                                for g in range(6 * hf, 6 * hf + 6):
                                    s_lo, s_hi = SLABS[(l, g)]
                                    for c in (0, 1):
                                        s0 = s_lo + 128 * c
                                        col0 = min(s0, SLOC - 128)
                                        nc.tensor.matmul(
                                            tps[:, 2 * g + c, :],
                                            qt_sb[g // 2][:, col0:col0 + 128],
                                            g_sb[:, g % 2, l, :],
                                            start=True, stop=True,
                                        )
                                th = tps[:, 12 * hf:12 * hf + 12, :]
                                # softmax over the dk axis
                                negmax = smp.tile([128, NH], F32, tag="nm",
                                                  name=f"nm{l}{hf}")
                                nc.vector.reduce_max(negmax, th,
                                                     axis=mybir.AxisListType.X,
                                                     negate=True)
                                av = smp.tile([128, NH, DK], F32, tag="A",
                                              name=f"A{l}{hf}")
                                nm_b = bass.AP(tensor=negmax.tensor,
                                               offset=negmax.offset,
                                               ap=[negmax.ap[0], negmax.ap[1],
                                                   [0, DK]])
                                nc.vector.tensor_add(av, th, nm_b)
                                nc.scalar.activation(
                                    av, av, mybir.ActivationFunctionType.Exp)
                                sm = smp.tile([128, NH], F32, tag="sm",
                                              name=f"sm{l}{hf}")
                                nc.vector.reduce_sum(sm, av,
                                                     axis=mybir.AxisListType.X)
                                inv = smp.tile([128, NH], F32, tag="inv",
                                               name=f"inv{l}{hf}")
                                nc.vector.reciprocal(inv, sm)
                                inv_b = bass.AP(tensor=inv.tensor, offset=inv.offset,
                                                ap=[inv.ap[0], inv.ap[1], [0, DK]])
                                nc.vector.tensor_mul(av, av, inv_b)

                                # transpose A chunks into M.T tiles (fp32r)
                                if stage >= 5:
                                    for g in range(6 * hf, 6 * hf + 6):
                                        s_lo, s_hi = SLABS[(l, g)]
                                        h2 = (g % 2) * 64
                                        for c in (0, 1):
                                            s0 = s_lo + 128 * c
                                            s1 = min(s_hi, s_lo + 128 * (c + 1))
                                            col0 = min(s0, SLOC - 128)
                                            dlt = s0 - col0
                                            trp = pptr.tile([128, 128], F32, tag="tr",
                                                            name=f"tr{l}{g}{c}")
                                            if h2 == 0:
                                                nc.tensor.transpose(
                                                    trp[0:64, :],
                                                    av[:, 2 * g + c - 12 * hf, :],
                                                    ident,
                                                )
                                                nc.any.tensor_copy(
                                                    mt_sb[g // 2][0:64, s0:s1],
                                                    trp[0:64, dlt:dlt + (s1 - s0)],
                                                )
                                            else:
                                                # transpose-mode psum out must
                                                # start at partition 0; emulate
                                                # via A.T @ I, emitting only the
                                                # valid output columns
                                                nc.tensor.matmul(
                                                    trp[64:128, 0:s1 - s0],
                                                    av[:, 2 * g + c - 12 * hf, :],
                                                    ident[:, dlt:dlt + (s1 - s0)],
                                                    start=True, stop=True,
                                                )
                                                nc.any.tensor_copy(
                                                    mt_sb[g // 2][64:128, s0:s1],
                                                    trp[64:128, 0:s1 - s0],
                                                )

                # output projections: O.T = Wo @ M, OUT.T = Wf @ O (fp32r)
                if stage >= 6:
                    with (
                        tc.tile_pool(name="ot", bufs=1) as otp,
                        tc.tile_pool(name="ppo", bufs=3, space="PSUM") as ppo,
                    ):
                        ot_sb = [otp.tile([128, SLOC], F32R, tag=f"ot{j}",
                                          name=f"ot{j}") for j in range(6)]
                        out_sb = [otp.tile([128, SLOC], F32, tag=f"ou{j}",
                                           name=f"ou{j}") for j in range(6)]
                        for j in range(6):
                            ps = ppo.tile([128, 512], F32, tag="po")
                            for k in range(6):
                                nc.tensor.matmul(
                                    ps, woT_sb[k][:, 128 * j:128 * (j + 1)], mt_sb[k],
                                    start=(k == 0), stop=(k == 5),
                                )
                            nc.scalar.activation(
                                ot_sb[j], ps, ACT_ID, bias=bias_sb[:, 3, j:j + 1],
                            )
                        for j in range(6):
                            ps = ppo.tile([128, 512], F32, tag="po")
                            for k in range(6):
                                nc.tensor.matmul(
                                    ps, wfT_sb[k][:, 128 * j:128 * (j + 1)], ot_sb[k],
                                    start=(k == 0), stop=(k == 5),
                                )
                            nc.scalar.activation(
                                out_sb[j], ps, ACT_ID, bias=bias_sb[:, 4, j:j + 1],
                            )
                            nc.sync.dma_start(out=outT[128 * j:128 * (j + 1), :],
                                              in_=out_sb[j])

    nc.finalize()
    return nc


_NC_CACHE = None


def make_in_maps(x, Wq, bq, Wk, bk, Wv, bv, Wo, bo, Wf, bf):
    xf = np.ascontiguousarray(np.asarray(x, np.float32).reshape(B * S, D))
    shared = {
        "wqT": _round_fp32r(np.asarray(Wq, np.float32).T),
        "wkT": _round_fp32r(np.asarray(Wk, np.float32).T),
        "wvT": _round_fp32r(np.asarray(Wv, np.float32).T),
        "woT": _round_fp32r(np.asarray(Wo, np.float32).T),
        "wfT": _round_fp32r(np.asarray(Wf, np.float32).T),
        "bias_po": np.stack(
            [np.asarray(b, np.float32).reshape(6, 128).T
             for b in (bq, bk, bv, bo, bf)],
            axis=1,
        ).copy(),
    }
    in_maps = []
    for c in range(NCORES):
        m = dict(shared)
        m["xT"] = _round_fp32r(xf[SLOC * c:SLOC * (c + 1), :].T)
        in_maps.append(m)
    return in_maps


def kernel(**inputs):
    global _NC_CACHE
    if _NC_CACHE is None:
        _NC_CACHE = build_nc()
    nc = _NC_CACHE
    in_maps = make_in_maps(**inputs)
    res = run_bass_kernel_spmd(nc, in_maps, list(range(NCORES)))
    out = np.empty((B * S, D), np.float32)
    for c in range(NCORES):
        out[SLOC * c:SLOC * (c + 1), :] = res.results[c]["outT"].T
    return out.reshape(B, S, D)
